# revision 1
# baseline (speedup 1.0000x reference)
"""JacobiKANLinear TRN2 Bass kernel.

out = silu(x) @ W_base^T + einsum('bik,oik->bo', P(tanh(x)), C) + bias

Host-side algebra: Jacobi polynomials (A=B=1, degree 5) are re-expressed in
the monomial basis.  D[o,i,j] = sum_k C[o,i,k] * T[k,j] where T holds the
monomial coefficients of P_k.  The j=0 term is constant (t^0 == 1) and folds
into the bias.  The device then computes 6 feature blocks
[silu(x), t, t^2, t^3, t^4, t^5] (t = tanh(x)) and one fused matmul with
contraction 6*1024 = 6144, plus a K=1 ones-x-bias matmul that initialises
PSUM with the bias broadcast.

Sharding (8 cores): 4 batch groups x 2 out-feature halves.  Per core:
batch shard 2048 rows, out shard 512 cols.  Matmuls run in float32r
(1 cycle/row on the PE at N=512 — bf16 speed with ~15x better accuracy).
"""
import numpy as np

import concourse.bass as bass
import concourse.mybir as mybir
import concourse.tile as tile
from concourse import bacc
from concourse.bass_utils import run_bass_kernel_spmd

BATCH = 8192
IN_F = 1024
OUT_F = 1024
DEGREE = 5
A = 1.0
B = 1.0

N_CORES = 8
BATCH_GROUPS = 4
OUT_HALVES = 2
B_SHARD = BATCH // BATCH_GROUPS        # 2048
O_SHARD = OUT_F // OUT_HALVES          # 512
N_BLOCKS = DEGREE + 1                  # 6 feature blocks
N_KT = N_BLOCKS * IN_F // 128          # 48 contraction tiles of 128
N_CHUNKS = B_SHARD // 128              # 16 batch chunks per core
IT_PER_BLOCK = IN_F // 128             # 8 in-feature tiles per block

F32 = mybir.dt.float32
F32R = mybir.dt.float32r


def _jacobi_monomial_matrix():
    """T[k, j] = coefficient of t^j in P_k (A=B=1), float64."""
    T = np.zeros((DEGREE + 1, DEGREE + 1), dtype=np.float64)
    polys = [np.zeros(DEGREE + 1) for _ in range(DEGREE + 1)]
    polys[0][0] = 1.0
    if DEGREE >= 1:
        # 0.5 * (2(A+1) t + (A-B))
        polys[1][1] = A + 1.0
        polys[1][0] = 0.5 * (A - B)
    for k in range(2, DEGREE + 1):
        alpha_n = 2.0 * k * (k + A + B) * (2 * k + A + B - 2)
        beta_n = (2 * k + A + B - 1) * (A ** 2 - B ** 2)
        gamma_n = (2 * k + A + B - 2) * (2 * k + A + B - 1) * (2 * k + A + B)
        delta_n = 2.0 * (k + A - 1) * (k + B - 1) * (2 * k + A + B)
        # P_k = ((beta + alpha t)/gamma) P_{k-1} - (delta/gamma) P_{k-2}
        p = np.zeros(DEGREE + 1)
        p += (beta_n / gamma_n) * polys[k - 1]
        p[1:] += (alpha_n / gamma_n) * polys[k - 1][:-1]
        p -= (delta_n / gamma_n) * polys[k - 2]
        polys[k] = p
    for k in range(DEGREE + 1):
        T[k] = polys[k]
    return T


def _build_nc():
    nc = bacc.Bacc()
    xt_in = nc.declare_dram_parameter(
        "xt", [IT_PER_BLOCK, 128, B_SHARD], F32, isOutput=False)
    w_in = nc.declare_dram_parameter(
        "w", [128, N_KT, O_SHARD], F32R, isOutput=False)
    bias_in = nc.declare_dram_parameter("biasv", [1, O_SHARD], F32R, isOutput=False)
    ones_in = nc.declare_dram_parameter("onesv", [1, 128], F32R, isOutput=False)
    out = nc.declare_dram_parameter("out", [B_SHARD, O_SHARD], F32, isOutput=True)

    with tile.TileContext(nc) as tc:
        with tc.tile_pool(name="wpool", bufs=1) as wpool, \
             tc.tile_pool(name="xpool", bufs=3) as xpool, \
             tc.tile_pool(name="fpool", bufs=2) as fpool, \
             tc.tile_pool(name="opool", bufs=3) as opool, \
             tc.tile_pool(name="psum", bufs=4, space="PSUM") as psum_pool:
            # Resident weights: one DMA per contraction tile so chunk-0
            # matmuls can start as slices land.
            w_sb = wpool.tile([128, N_KT, O_SHARD], F32R)
            for kt in range(N_KT):
                nc.sync.dma_start(out=w_sb[:, kt, :], in_=w_in[:, kt, :])
            bias_sb = wpool.tile([1, O_SHARD], F32R)
            nc.sync.dma_start(out=bias_sb[:], in_=bias_in[:])
            ones_sb = wpool.tile([1, 128], F32R)
            nc.sync.dma_start(out=ones_sb[:], in_=ones_in[:])

            for m in range(N_CHUNKS):
                bsl = bass.ts(m, 128)
                x_m = xpool.tile([128, IT_PER_BLOCK, 128], F32, tag="x")
                nc.sync.dma_start(
                    out=x_m[:], in_=xt_in[:, :, bsl].transpose([1, 0, 2]))

                silu_m = fpool.tile([128, IT_PER_BLOCK, 128], F32R, tag="silu")
                t_m = fpool.tile([128, IT_PER_BLOCK, 128], F32R, tag="t1")
                nc.scalar.activation(
                    silu_m[:], x_m[:], mybir.ActivationFunctionType.Silu)
                nc.scalar.activation(
                    t_m[:], x_m[:], mybir.ActivationFunctionType.Tanh)
                t2_m = fpool.tile([128, IT_PER_BLOCK, 128], F32R, tag="t2")
                nc.vector.tensor_mul(t2_m[:], t_m[:], t_m[:])
                t3_m = fpool.tile([128, IT_PER_BLOCK, 128], F32R, tag="t3")
                nc.vector.tensor_mul(t3_m[:], t2_m[:], t_m[:])
                t4_m = fpool.tile([128, IT_PER_BLOCK, 128], F32R, tag="t4")
                nc.vector.tensor_mul(t4_m[:], t3_m[:], t_m[:])
                t5_m = fpool.tile([128, IT_PER_BLOCK, 128], F32R, tag="t5")
                nc.vector.tensor_mul(t5_m[:], t4_m[:], t_m[:])
                blocks = [silu_m, t_m, t2_m, t3_m, t4_m, t5_m]

                ps = psum_pool.tile([128, O_SHARD], F32, tag="ps")
                # PSUM init: every row gets the bias vector (ones.T @ bias).
                nc.tensor.matmul(
                    ps[:], ones_sb[:], bias_sb[:], start=True, stop=False)
                for b in range(N_BLOCKS):
                    for it in range(IT_PER_BLOCK):
                        kt = b * IT_PER_BLOCK + it
                        nc.tensor.matmul(
                            ps[:], blocks[b][:, it, :], w_sb[:, kt, :],
                            start=False, stop=(kt == N_KT - 1))
                o_m = opool.tile([128, O_SHARD], F32, tag="o")
                nc.vector.tensor_copy(o_m[:], ps[:])
                nc.sync.dma_start(out=out[bsl, :], in_=o_m[:])
    nc.finalize()
    return nc


_NC_CACHE = None


def _get_nc():
    global _NC_CACHE
    if _NC_CACHE is None:
        _NC_CACHE = _build_nc()
    return _NC_CACHE


def _prepare_host(x, base_weight, jacobi_coeffs, bias):
    T = _jacobi_monomial_matrix()
    D = np.einsum("oik,kj->oij", jacobi_coeffs.astype(np.float64), T)
    bias_eff = bias.astype(np.float64) + D[:, :, 0].sum(axis=1)

    # W'[f, o]: 6 blocks of IN_F feature rows: silu -> base_weight, t^j -> D_j
    w_full = np.empty((N_BLOCKS * IN_F, OUT_F), dtype=np.float32)
    w_full[0:IN_F] = base_weight.T
    for j in range(1, N_BLOCKS):
        w_full[j * IN_F:(j + 1) * IN_F] = D[:, :, j].T.astype(np.float32)

    w_halves = []
    bias_halves = []
    for h in range(OUT_HALVES):
        wh = w_full[:, h * O_SHARD:(h + 1) * O_SHARD]
        # SBUF layout [128, N_KT, O_SHARD]: [p, kt, n] = wh[kt*128 + p, n]
        wh = np.ascontiguousarray(
            wh.reshape(N_KT, 128, O_SHARD).transpose(1, 0, 2))
        w_halves.append(wh)
        bias_halves.append(np.ascontiguousarray(
            bias_eff[h * O_SHARD:(h + 1) * O_SHARD].astype(np.float32)[None, :]))

    xt_groups = []
    for g in range(BATCH_GROUPS):
        xs = x[g * B_SHARD:(g + 1) * B_SHARD]              # (B_SHARD, IN_F)
        # [it, p, b] = xs[b, it*128 + p]
        xt = np.ascontiguousarray(xs.T.reshape(IT_PER_BLOCK, 128, B_SHARD))
        xt_groups.append(xt)
    return xt_groups, w_halves, bias_halves


def kernel(x, base_weight, jacobi_coeffs, bias):
    x = np.asarray(x, dtype=np.float32)
    base_weight = np.asarray(base_weight, dtype=np.float32)
    jacobi_coeffs = np.asarray(jacobi_coeffs, dtype=np.float32)
    bias = np.asarray(bias, dtype=np.float32)

    xt_groups, w_halves, bias_halves = _prepare_host(
        x, base_weight, jacobi_coeffs, bias)

    in_maps = []
    for c in range(N_CORES):
        g, h = c // OUT_HALVES, c % OUT_HALVES
        in_maps.append({
            "xt": xt_groups[g],
            "w": w_halves[h],
            "biasv": bias_halves[h],
            "onesv": np.ones((1, 128), dtype=np.float32),
        })

    nc = _get_nc()
    res = run_bass_kernel_spmd(nc, in_maps, core_ids=list(range(N_CORES)))

    out = np.empty((BATCH, OUT_F), dtype=np.float32)
    for c in range(N_CORES):
        g, h = c // OUT_HALVES, c % OUT_HALVES
        out[g * B_SHARD:(g + 1) * B_SHARD,
            h * O_SHARD:(h + 1) * O_SHARD] = res.results[c]["out"]
    return out



# revision 3
# speedup vs baseline: 1.1967x; 1.1967x over previous
"""JacobiKANLinear TRN2 Bass kernel.

out = silu(x) @ W_base^T + einsum('bik,oik->bo', P(tanh(x)), C) + bias

Host-side algebra: Jacobi polynomials (A=B=1, degree 5) are re-expressed in
the monomial basis.  D[o,i,j] = sum_k C[o,i,k] * T[k,j] where T holds the
monomial coefficients of P_k.  The j=0 term is constant (t^0 == 1) and folds
into the bias.  The device then computes 6 feature blocks
[silu(x), t, t^2, t^3, t^4, t^5] (t = tanh(x)) and one fused matmul with
contraction 6*1024 = 6144.  Bias is added during the PSUM->SBUF drain via a
host-broadcast [128, O_SHARD] tile (no PSUM-init matmul).

Sharding (8 cores): 4 batch groups x 2 out-feature halves.  Per core:
batch shard 2048 rows, out shard 512 cols.  Matmuls run in float32r
(1 cycle/row on the PE at N=512 — bf16 speed with ~15x better accuracy).

Scheduling: the v1 kernel spent its first 45us with the PE idle — the
12.6MB weight DMA plus bias/ones DMAs (emitted last) gated every matmul.
v2 overlaps the weight-DMA phase with compute: the first 4 batch chunks'
feature blocks are built immediately (their x DMAs are interleaved into the
weight stream), and phase A issues matmuls kt-round-robin across those 4
chunks so every landing weight tile immediately feeds 4 matmuls (~850ns of
PE work per ~790ns DMA).  Phase B (chunks 4..15) then streams dense.
"""
import numpy as np

import concourse.bass as bass
import concourse.mybir as mybir
import concourse.tile as tile
from concourse import bacc
from concourse.bass_utils import run_bass_kernel_spmd

BATCH = 8192
IN_F = 1024
OUT_F = 1024
DEGREE = 5
A = 1.0
B = 1.0

N_CORES = 8
BATCH_GROUPS = 4
OUT_HALVES = 2
B_SHARD = BATCH // BATCH_GROUPS        # 2048
O_SHARD = OUT_F // OUT_HALVES          # 512
N_BLOCKS = DEGREE + 1                  # 6 feature blocks
N_KT = N_BLOCKS * IN_F // 128          # 48 contraction tiles of 128
N_CHUNKS = B_SHARD // 128              # 16 batch chunks per core
IT_PER_BLOCK = IN_F // 128             # 8 in-feature tiles per block
N_WARM = 4                             # chunks interleaved with the w DMA

F32 = mybir.dt.float32
F32R = mybir.dt.float32r


def _jacobi_monomial_matrix():
    """T[k, j] = coefficient of t^j in P_k (A=B=1), float64."""
    T = np.zeros((DEGREE + 1, DEGREE + 1), dtype=np.float64)
    polys = [np.zeros(DEGREE + 1) for _ in range(DEGREE + 1)]
    polys[0][0] = 1.0
    if DEGREE >= 1:
        # 0.5 * (2(A+1) t + (A-B))
        polys[1][1] = A + 1.0
        polys[1][0] = 0.5 * (A - B)
    for k in range(2, DEGREE + 1):
        alpha_n = 2.0 * k * (k + A + B) * (2 * k + A + B - 2)
        beta_n = (2 * k + A + B - 1) * (A ** 2 - B ** 2)
        gamma_n = (2 * k + A + B - 2) * (2 * k + A + B - 1) * (2 * k + A + B)
        delta_n = 2.0 * (k + A - 1) * (k + B - 1) * (2 * k + A + B)
        # P_k = ((beta + alpha t)/gamma) P_{k-1} - (delta/gamma) P_{k-2}
        p = np.zeros(DEGREE + 1)
        p += (beta_n / gamma_n) * polys[k - 1]
        p[1:] += (alpha_n / gamma_n) * polys[k - 1][:-1]
        p -= (delta_n / gamma_n) * polys[k - 2]
        polys[k] = p
    for k in range(DEGREE + 1):
        T[k] = polys[k]
    return T


def _build_nc():
    nc = bacc.Bacc()
    xt_in = nc.declare_dram_parameter(
        "xt", [IT_PER_BLOCK, 128, B_SHARD], F32, isOutput=False)
    w_in = nc.declare_dram_parameter(
        "w", [128, N_KT, O_SHARD], F32R, isOutput=False)
    biasb_in = nc.declare_dram_parameter(
        "biasb", [128, O_SHARD], F32, isOutput=False)
    out = nc.declare_dram_parameter("out", [B_SHARD, O_SHARD], F32, isOutput=True)

    with tile.TileContext(nc) as tc:
        with tc.tile_pool(name="wpool", bufs=1) as wpool, \
             tc.tile_pool(name="xpool", bufs=2) as xpool, \
             tc.tile_pool(name="fpool", bufs=N_WARM) as fpool, \
             tc.tile_pool(name="opool", bufs=2) as opool, \
             tc.tile_pool(name="psum", bufs=8, space="PSUM") as psum_pool:

            def x_dma(m):
                x_m = xpool.tile([128, IT_PER_BLOCK, 128], F32, tag="x")
                nc.sync.dma_start(
                    out=x_m[:], in_=xt_in[:, :, bass.ts(m, 128)].transpose([1, 0, 2]))
                return x_m

            def blocks_for(x_m):
                silu_m = fpool.tile([128, IT_PER_BLOCK, 128], F32R, tag="silu")
                t_m = fpool.tile([128, IT_PER_BLOCK, 128], F32R, tag="t1")
                nc.scalar.activation(
                    silu_m[:], x_m[:], mybir.ActivationFunctionType.Silu)
                nc.scalar.activation(
                    t_m[:], x_m[:], mybir.ActivationFunctionType.Tanh)
                t2_m = fpool.tile([128, IT_PER_BLOCK, 128], F32R, tag="t2")
                nc.vector.tensor_mul(t2_m[:], t_m[:], t_m[:])
                t3_m = fpool.tile([128, IT_PER_BLOCK, 128], F32R, tag="t3")
                nc.vector.tensor_mul(t3_m[:], t2_m[:], t_m[:])
                t4_m = fpool.tile([128, IT_PER_BLOCK, 128], F32R, tag="t4")
                nc.vector.tensor_mul(t4_m[:], t3_m[:], t_m[:])
                t5_m = fpool.tile([128, IT_PER_BLOCK, 128], F32R, tag="t5")
                nc.vector.tensor_mul(t5_m[:], t4_m[:], t_m[:])
                return [silu_m, t_m, t2_m, t3_m, t4_m, t5_m]

            def drain(m, ps):
                o_m = opool.tile([128, O_SHARD], F32, tag="o")
                nc.vector.tensor_add(o_m[:], ps[:], biasb_sb[:])
                nc.sync.dma_start(out=out[bass.ts(m, 128), :], in_=o_m[:])

            # --- DMA issue order: bias first, first x chunks interleaved
            # into the weight stream so phase-A blocks are ready early.
            biasb_sb = wpool.tile([128, O_SHARD], F32)
            nc.sync.dma_start(out=biasb_sb[:], in_=biasb_in[:])

            w_sb = wpool.tile([128, N_KT, O_SHARD], F32R)
            x_tiles = {}
            x_tiles[0] = x_dma(0)
            x_tiles[1] = x_dma(1)
            for kt in range(4):
                nc.sync.dma_start(out=w_sb[:, kt, :], in_=w_in[:, kt, :])
            x_tiles[2] = x_dma(2)
            for kt in range(4, 8):
                nc.sync.dma_start(out=w_sb[:, kt, :], in_=w_in[:, kt, :])
            x_tiles[3] = x_dma(3)
            for kt in range(8, N_KT):
                nc.sync.dma_start(out=w_sb[:, kt, :], in_=w_in[:, kt, :])

            # --- Phase A: 4 chunks, kt-round-robin matmuls.
            warm_blocks = []
            warm_ps = []
            for c in range(N_WARM):
                warm_blocks.append(blocks_for(x_tiles[c]))
                ps_c = psum_pool.tile([128, O_SHARD], F32, tag="ps")
                warm_ps.append(ps_c)
            for kt in range(N_KT):
                b, it = kt // IT_PER_BLOCK, kt % IT_PER_BLOCK
                for c in range(N_WARM):
                    nc.tensor.matmul(
                        warm_ps[c][:], warm_blocks[c][b][:, it, :], w_sb[:, kt, :],
                        start=(kt == 0), stop=(kt == N_KT - 1))
            for c in range(N_WARM):
                drain(c, warm_ps[c])

            # --- Phase B: remaining chunks, chunk-major (weights resident).
            for m in range(N_WARM, N_CHUNKS):
                x_m = x_dma(m)
                blocks = blocks_for(x_m)
                ps = psum_pool.tile([128, O_SHARD], F32, tag="ps")
                for kt in range(N_KT):
                    b, it = kt // IT_PER_BLOCK, kt % IT_PER_BLOCK
                    nc.tensor.matmul(
                        ps[:], blocks[b][:, it, :], w_sb[:, kt, :],
                        start=(kt == 0), stop=(kt == N_KT - 1))
                drain(m, ps)
    nc.finalize()
    return nc


_NC_CACHE = None


def _get_nc():
    global _NC_CACHE
    if _NC_CACHE is None:
        _NC_CACHE = _build_nc()
    return _NC_CACHE


def _prepare_host(x, base_weight, jacobi_coeffs, bias):
    T = _jacobi_monomial_matrix()
    D = np.einsum("oik,kj->oij", jacobi_coeffs.astype(np.float64), T)
    bias_eff = bias.astype(np.float64) + D[:, :, 0].sum(axis=1)

    # W'[f, o]: 6 blocks of IN_F feature rows: silu -> base_weight, t^j -> D_j
    w_full = np.empty((N_BLOCKS * IN_F, OUT_F), dtype=np.float32)
    w_full[0:IN_F] = base_weight.T
    for j in range(1, N_BLOCKS):
        w_full[j * IN_F:(j + 1) * IN_F] = D[:, :, j].T.astype(np.float32)

    w_halves = []
    biasb_halves = []
    for h in range(OUT_HALVES):
        wh = w_full[:, h * O_SHARD:(h + 1) * O_SHARD]
        # SBUF layout [128, N_KT, O_SHARD]: [p, kt, n] = wh[kt*128 + p, n]
        wh = np.ascontiguousarray(
            wh.reshape(N_KT, 128, O_SHARD).transpose(1, 0, 2))
        w_halves.append(wh)
        bh = bias_eff[h * O_SHARD:(h + 1) * O_SHARD].astype(np.float32)
        biasb_halves.append(np.ascontiguousarray(
            np.broadcast_to(bh[None, :], (128, O_SHARD))))

    xt_groups = []
    for g in range(BATCH_GROUPS):
        xs = x[g * B_SHARD:(g + 1) * B_SHARD]              # (B_SHARD, IN_F)
        # [it, p, b] = xs[b, it*128 + p]
        xt = np.ascontiguousarray(xs.T.reshape(IT_PER_BLOCK, 128, B_SHARD))
        xt_groups.append(xt)
    return xt_groups, w_halves, biasb_halves


def _in_maps(x, base_weight, jacobi_coeffs, bias):
    xt_groups, w_halves, biasb_halves = _prepare_host(
        x, base_weight, jacobi_coeffs, bias)
    maps = []
    for c in range(N_CORES):
        g, h = c // OUT_HALVES, c % OUT_HALVES
        maps.append({
            "xt": xt_groups[g],
            "w": w_halves[h],
            "biasb": biasb_halves[h],
        })
    return maps


def kernel(x, base_weight, jacobi_coeffs, bias):
    x = np.asarray(x, dtype=np.float32)
    base_weight = np.asarray(base_weight, dtype=np.float32)
    jacobi_coeffs = np.asarray(jacobi_coeffs, dtype=np.float32)
    bias = np.asarray(bias, dtype=np.float32)

    in_maps = _in_maps(x, base_weight, jacobi_coeffs, bias)
    nc = _get_nc()
    res = run_bass_kernel_spmd(nc, in_maps, core_ids=list(range(N_CORES)))

    out = np.empty((BATCH, OUT_F), dtype=np.float32)
    for c in range(N_CORES):
        g, h = c // OUT_HALVES, c % OUT_HALVES
        out[g * B_SHARD:(g + 1) * B_SHARD,
            h * O_SHARD:(h + 1) * O_SHARD] = res.results[c]["out"]
    return out
